# revision 1
# baseline (speedup 1.0000x reference)
"""MixedEmbeddingV2 Trainium2 kernel.

out[b, s, :] = emb_weight[x[b, s], :] * col_scale
  col_scale[j] = sum_i weights[i] * [j < dims_i],  dims = (192, 384, 576, 768)

Sharding: token-parallel across 8 cores (batch row b -> core b), table
replicated per core. No collectives. Per core: 16 indirect-DMA row gathers
of [128, 768] f32, DVE column-scale multiply, contiguous write-back.

Raw Bass (not Tile): the DVE TensorTensor encoding on TRN2 rejects multiple
attached sync waits, so all cross-engine sync is standalone wait_ge
instructions with one semaphore per producer stream.
"""

import numpy as np

VOCAB = 50257
D = 768
B, S = 8, 2048
N_CORES = 8
TOK = (B * S) // N_CORES  # 2048 tokens per core
NT = TOK // 128           # 16 gather tiles per core
DIMS = (192, 384, 576, 768)

_cache = {}


def _build_nc(R=1):
    # R = benchmark repeat count: the pipeline body runs R times inside one
    # NEFF (R>1 reuses tiles with slot-recycle waits). Grading uses R=1.
    import concourse.bass as bass
    import concourse.mybir as mybir
    from contextlib import ExitStack

    f32 = mybir.dt.float32
    i32 = mybir.dt.int32

    nc = bass.Bass()
    x_h = nc.declare_dram_parameter("x_idx", [128, NT], i32, isOutput=False)
    s_h = nc.declare_dram_parameter("col_scale", [128, D], f32, isOutput=False)
    t_h = nc.declare_dram_parameter("emb", [VOCAB, D], f32, isOutput=False)
    o_h = nc.declare_dram_parameter("out", [TOK, D], f32, isOutput=True)

    with ExitStack() as es:
        idx = es.enter_context(nc.sbuf_tensor("idx", [128, NT], i32))
        scale = es.enter_context(nc.sbuf_tensor("scale", [128, D], f32))
        gts = [
            es.enter_context(nc.sbuf_tensor(f"gt{g}", [128, D], f32))
            for g in range(NT)
        ]
        i_sem = es.enter_context(nc.semaphore("i_sem"))
        s_sem = es.enter_context(nc.semaphore("s_sem"))
        g_sems = [
            es.enter_context(nc.semaphore(f"g_sem{g}")) for g in range(NT)
        ]
        m_sem = es.enter_context(nc.semaphore("m_sem"))
        o_sem = es.enter_context(nc.semaphore("o_sem"))

        with nc.Block() as block:

            @block.sync
            def _(sync: bass.BassEngine):
                sync.dma_start(out=idx[:], in_=x_h[:]).then_inc(i_sem, 16)
                sync.dma_start(out=scale[:], in_=s_h[:]).then_inc(s_sem, 16)
                # end-of-kernel drain: all output stores landed
                sync.wait_ge(o_sem, 16 * NT * R)

            @block.gpsimd
            def _(gp: bass.BassEngine):
                gp.wait_ge(i_sem, 16)
                for r in range(R):
                    for g in range(NT):
                        if r > 0:
                            # slot recycle: round r-1's store of this tile
                            # must have drained before regathering into it
                            gp.wait_ge(o_sem, 16 * (NT * (r - 1) + g + 1))
                        gp.indirect_dma_start(
                            out=gts[g][:],
                            out_offset=None,
                            in_=t_h[:],
                            in_offset=bass.IndirectOffsetOnAxis(
                                ap=idx[:, g : g + 1], axis=0
                            ),
                        ).then_inc(g_sems[g], 16)

            @block.vector
            def _(v: bass.BassEngine):
                v.wait_ge(s_sem, 16)
                for r in range(R):
                    for g in range(NT):
                        v.wait_ge(g_sems[g], 16 * (r + 1))
                        v.tensor_mul(
                            out=gts[g][:], in0=gts[g][:], in1=scale[:]
                        ).then_inc(m_sem, 1)

            @block.scalar
            def _(sc: bass.BassEngine):
                for r in range(R):
                    for g in range(NT):
                        sc.wait_ge(m_sem, NT * r + g + 1)
                        sc.dma_start(
                            out=o_h[g * 128 : (g + 1) * 128, :], in_=gts[g][:]
                        ).then_inc(o_sem, 16)

    return nc


def _get_nc(R=1):
    key = ("nc", R)
    if key not in _cache:
        _cache[key] = _build_nc(R)
    return _cache[key]


def _make_in_maps(x, weights, emb_weight):
    weights = np.asarray(weights, dtype=np.float32)
    emb = np.ascontiguousarray(np.asarray(emb_weight, dtype=np.float32))

    col = np.arange(D)
    mask = (col[None, :] < np.asarray(DIMS)[:, None]).astype(np.float32)
    col_scale = (weights @ mask).astype(np.float32)  # [D]
    scale_bcast = np.ascontiguousarray(np.broadcast_to(col_scale, (128, D)))

    x32 = np.asarray(x).reshape(N_CORES, TOK).astype(np.int32)
    in_maps = []
    for c in range(N_CORES):
        # SBUF idx tile [p, g] holds token g*128+p of this core's shard.
        xi = np.ascontiguousarray(x32[c].reshape(NT, 128).T)
        in_maps.append({"x_idx": xi, "col_scale": scale_bcast, "emb": emb})
    return in_maps


def _run(x, weights, emb_weight, **spmd_kwargs):
    from concourse.bass_utils import run_bass_kernel_spmd

    in_maps = _make_in_maps(x, weights, emb_weight)
    nc = _get_nc()
    res = run_bass_kernel_spmd(nc, in_maps, list(range(N_CORES)), **spmd_kwargs)
    out = np.stack([res.results[c]["out"] for c in range(N_CORES)], axis=0)
    return out.reshape(B, S, D), res


def kernel(x, weights, emb_weight):
    out, _ = _run(x, weights, emb_weight)
    return out



# revision 5
# speedup vs baseline: 609.5455x; 609.5455x over previous
"""MixedEmbeddingV2 Trainium2 kernel.

out[b, s, :] = emb_weight[x[b, s], :] * col_scale
  col_scale[j] = sum_i weights[i] * [j < dims_i],  dims = (192, 384, 576, 768)

Sharding: token-parallel across 8 cores (batch row b -> core b), table kept
in DRAM per core (no preload). Per core: the 2048-row gather runs through
the SWDGE dma_gather ucode (one 3KB descriptor per row, spread over the 16
SDMA rings) instead of the qPoolDynamic indirect-DMA path, which processes
the same gather ~500x slower.

dma_gather indices are int16, so the 50257-row table is covered with two
clamped passes: pass-lo gathers min(x, 32767) from the full table, pass-hi
gathers max(x - 32768, 0) from a +32768-row base-offset view. DVE then
combines the two buffers with premultiplied select masks that also fold in
col_scale:  out = lo * sel_lo + hi * sel_hi, where sel_lo[tok, j] =
col_scale[j] * [x_tok < 32768] and sel_hi its complement.

Work is chunked 4x512 tokens so gathers, DVE combine, and output stores
pipeline across engines (Pool / DVE / Act).
"""

import numpy as np

VOCAB = 50257
D = 768
B, S = 8, 2048
N_CORES = 8
TOK = (B * S) // N_CORES  # 2048 tokens per core
SPLIT = 32768             # int16 index limit boundary
HI_ROWS = VOCAB - SPLIT   # 17489
DIMS = (192, 384, 576, 768)

NCHUNK = 4
CH = TOK // NCHUNK        # 512 tokens per chunk
SLOTS = TOK // 128        # 16 output slots of [128, 768]
CSLOT = CH // 128         # 4 slots per chunk
ICOL = TOK // 16          # 128 idx-tile columns
CICOL = CH // 16          # 32 idx-tile columns per chunk

_cache = {}


def _build_nc(R=1):
    # R = benchmark repeat count: the pipeline body runs R times inside one
    # NEFF (R>1 reuses tiles with slot-recycle waits). Grading uses R=1.
    import concourse.bass as bass
    import concourse.mybir as mybir
    from concourse.library_config import mlp
    from contextlib import ExitStack

    f32 = mybir.dt.float32
    i16 = mybir.dt.int16

    nc = bass.Bass()
    xlo_h = nc.declare_dram_parameter("x_lo", [128, ICOL], i16, isOutput=False)
    xhi_h = nc.declare_dram_parameter("x_hi", [128, ICOL], i16, isOutput=False)
    slo_h = nc.declare_dram_parameter("sel_lo", [128, SLOTS * D], f32, isOutput=False)
    shi_h = nc.declare_dram_parameter("sel_hi", [128, SLOTS * D], f32, isOutput=False)
    t_h = nc.declare_dram_parameter("emb", [VOCAB, D], f32, isOutput=False)
    o_h = nc.declare_dram_parameter("out", [TOK, D], f32, isOutput=True)

    with ExitStack() as es:
        ilo = es.enter_context(nc.sbuf_tensor("ilo", [128, ICOL], i16))
        ihi = es.enter_context(nc.sbuf_tensor("ihi", [128, ICOL], i16))
        slo = es.enter_context(nc.sbuf_tensor("slo", [128, SLOTS, D], f32))
        shi = es.enter_context(nc.sbuf_tensor("shi", [128, SLOTS, D], f32))
        blo = es.enter_context(nc.sbuf_tensor("blo", [128, SLOTS, D], f32))
        bhi = es.enter_context(nc.sbuf_tensor("bhi", [128, SLOTS, D], f32))
        ld_sem = es.enter_context(nc.semaphore("ld_sem"))
        glo_sem = es.enter_context(nc.semaphore("glo_sem"))
        ghi_sem = es.enter_context(nc.semaphore("ghi_sem"))
        m_sem = es.enter_context(nc.semaphore("m_sem"))
        o_sem = es.enter_context(nc.semaphore("o_sem"))

        with nc.Block() as block:

            @block.sync
            def _(sync: bass.BassEngine):
                sync.dma_start(out=ilo[:], in_=xlo_h[:]).then_inc(ld_sem, 16)
                sync.dma_start(out=ihi[:], in_=xhi_h[:]).then_inc(ld_sem, 16)
                sync.dma_start(out=slo[:], in_=slo_h[:]).then_inc(ld_sem, 16)
                sync.dma_start(out=shi[:], in_=shi_h[:]).then_inc(ld_sem, 16)
                # end-of-kernel drain: all output stores landed
                sync.wait_ge(o_sem, 16 * NCHUNK * R)

            @block.gpsimd
            def _(gp: bass.BassGpSimd):
                gp.load_library(mlp)
                gp.wait_ge(ld_sem, 64)
                # one shared count register; a fresh to_reg per gather
                # exhausts the Pool register file at R=100
                ch_reg = gp.to_reg(CH)
                for r in range(R):
                    for k in range(NCHUNK):
                        if r > 0:
                            # slot recycle: round r-1's store of this chunk
                            # must have drained before regathering into it
                            gp.wait_ge(o_sem, 16 * (NCHUNK * (r - 1) + k + 1))
                        gp.dma_gather(
                            blo[:, k * CSLOT : (k + 1) * CSLOT, :],
                            t_h[:],
                            ilo[:, k * CICOL : (k + 1) * CICOL],
                            CH,
                            ch_reg,
                            D,
                        ).then_inc(glo_sem, 16)
                        gp.dma_gather(
                            bhi[:, k * CSLOT : (k + 1) * CSLOT, :],
                            t_h[SPLIT:, :],
                            ihi[:, k * CICOL : (k + 1) * CICOL],
                            CH,
                            ch_reg,
                            D,
                        ).then_inc(ghi_sem, 16)

            @block.vector
            def _(v: bass.BassEngine):
                v.wait_ge(ld_sem, 64)
                for r in range(R):
                    for k in range(NCHUNK):
                        n = NCHUNK * r + k + 1
                        lo_c = blo[:, k * CSLOT : (k + 1) * CSLOT, :]
                        hi_c = bhi[:, k * CSLOT : (k + 1) * CSLOT, :]
                        v.wait_ge(glo_sem, 16 * n)
                        v.tensor_mul(
                            out=lo_c,
                            in0=lo_c,
                            in1=slo[:, k * CSLOT : (k + 1) * CSLOT, :],
                        )
                        v.wait_ge(ghi_sem, 16 * n)
                        v.tensor_mul(
                            out=hi_c,
                            in0=hi_c,
                            in1=shi[:, k * CSLOT : (k + 1) * CSLOT, :],
                        )
                        v.tensor_add(out=lo_c, in0=lo_c, in1=hi_c).then_inc(
                            m_sem, 1
                        )

            @block.scalar
            def _(sc: bass.BassEngine):
                for r in range(R):
                    for k in range(NCHUNK):
                        sc.wait_ge(m_sem, NCHUNK * r + k + 1)
                        sc.dma_start(
                            out=o_h[k * CH : (k + 1) * CH, :].rearrange(
                                "(c p) j -> p c j", p=128
                            ),
                            in_=blo[:, k * CSLOT : (k + 1) * CSLOT, :],
                        ).then_inc(o_sem, 16)

    # Raw Bass skips Bacc's codegen pass, leaving extended-inst encodings
    # (load_library's ModifyPoolConfig) empty -> walrus "ISA wrong length".
    mybir.codegen_inst_isa_subclasses(nc)
    return nc


def _get_nc(R=1):
    key = ("nc", R)
    if key not in _cache:
        _cache[key] = _build_nc(R)
    return _cache[key]


def _idx_tile(v):
    # dma_gather idx layout: token i lives at partition i % 16, column
    # i // 16; the 16-partition pattern is replicated 8x so each Q7 cpu
    # pair reads its own partition stripe.
    t = np.asarray(v, dtype=np.int16).reshape(ICOL, 16).T  # [16, ICOL]
    return np.ascontiguousarray(np.tile(t, (8, 1)))  # [128, ICOL]


def _make_in_maps(x, weights, emb_weight):
    weights = np.asarray(weights, dtype=np.float32)
    emb = np.ascontiguousarray(np.asarray(emb_weight, dtype=np.float32))

    col = np.arange(D)
    mask = (col[None, :] < np.asarray(DIMS)[:, None]).astype(np.float32)
    col_scale = (weights @ mask).astype(np.float32)  # [D]

    x32 = np.asarray(x).reshape(N_CORES, TOK).astype(np.int32)
    in_maps = []
    for c in range(N_CORES):
        xc = x32[c]
        lo = np.minimum(xc, SPLIT - 1)
        hi = np.maximum(xc - SPLIT, 0)
        # sel tiles follow the gather output layout: token t -> partition
        # t % 128, slot t // 128.
        is_lo = (xc < SPLIT).astype(np.float32).reshape(SLOTS, 128).T  # [p, c]
        sel_lo = is_lo[:, :, None] * col_scale[None, None, :]
        sel_hi = (1.0 - is_lo)[:, :, None] * col_scale[None, None, :]
        in_maps.append(
            {
                "x_lo": _idx_tile(lo),
                "x_hi": _idx_tile(hi),
                "sel_lo": np.ascontiguousarray(
                    sel_lo.reshape(128, SLOTS * D), dtype=np.float32
                ),
                "sel_hi": np.ascontiguousarray(
                    sel_hi.reshape(128, SLOTS * D), dtype=np.float32
                ),
                "emb": emb,
            }
        )
    return in_maps


def _run(x, weights, emb_weight, **spmd_kwargs):
    from concourse.bass_utils import run_bass_kernel_spmd

    in_maps = _make_in_maps(x, weights, emb_weight)
    nc = _get_nc()
    res = run_bass_kernel_spmd(nc, in_maps, list(range(N_CORES)), **spmd_kwargs)
    out = np.stack([res.results[c]["out"] for c in range(N_CORES)], axis=0)
    return out.reshape(B, S, D), res


def kernel(x, weights, emb_weight):
    out, _ = _run(x, weights, emb_weight)
    return out
